# revision 2
# baseline (speedup 1.0000x reference)
"""Trainium2 Bass kernel for the Clifford (geometric) product on Cl(3,0).

out[n, k] = sum_{i,j} S[i,j,k] * a[n,i] * b[n,j],  S = structure constants
(64 nonzeros, one per (i,j), signs +-1).

End-to-end wall time is dominated by host<->device transfer of the
operands, so the wire format is fp16: the host packs a and b into one
[n, 16] fp16 tensor (halves the bytes and pays the per-array transfer
fixed cost once), the device computes entirely in fp16 (2x DVE
throughput), and the fp16 result is widened back to f32 on the host.
Measured accuracy of this scheme vs the f32 reference: ~1e-3 max-rel.

Per NeuronCore (batch sharded 8 ways):
  - Tiles of 128 partitions x E multivectors/partition, natural
    interleaved layout [128, E*16] (contiguous DMA).
  - The 64 signed products are emitted by ~23 DVE ops (tensor_tensor /
    scalar_tensor_tensor) whose access patterns enumerate "affine boxes"
    of (i, j, output-slot) triples; signs fold into the STT immediate.
  - Products land grouped 8-per-output-component; the 8-way sums run as
    3-level trees, split between the Vector engine (k < KD) and GPSIMD
    (k >= KD) so both engines work in parallel.
"""

import os

# Whole-tile dependency tracking: the ~23 interleaved strided product writes
# per tile otherwise become per-subtile dep edges, whose un-coalesced sem
# waits overflow the ISA's per-instruction wait-command limit.
os.environ.setdefault("BY_DEFAULT_DISABLE_SUBTILE_DEPS", "1")

import numpy as np
from concurrent.futures import ThreadPoolExecutor
from itertools import combinations, permutations

import concourse.bass as bass
import concourse.bacc as bacc
import concourse.mybir as mybir
from concourse import bass_utils
from concourse.tile import TileContext

# ---------------------------------------------------------------- geometry
N_TOTAL = 4194304
N_CORES = 8
NC = N_TOTAL // N_CORES        # 524288 multivectors per core
P = 128                        # partitions
E = 256                        # multivectors per partition per tile
TILE_MV = P * E                # 32768
N_TILES = NC // TILE_MV        # 16
KD = 2                         # components 0..KD-1 reduced on DVE, rest GPSIMD

F16 = mybir.dt.float16
_POOL = ThreadPoolExecutor(max_workers=16)


# ------------------------------------------------- structure constants S
def _build_S():
    basis = [(), (0,), (1,), (2,), (0, 1), (0, 2), (1, 2), (0, 1, 2)]
    b2i = {b: i for i, b in enumerate(basis)}
    S = np.zeros((8, 8, 8), dtype=np.int32)
    for i, a in enumerate(basis):
        for j, b in enumerate(basis):
            comb = list(a) + list(b)
            sign = 1
            n = len(comb)
            for pn in range(n):
                for pos in range(n - 1 - pn):
                    if comb[pos] > comb[pos + 1]:
                        comb[pos], comb[pos + 1] = comb[pos + 1], comb[pos]
                        sign *= -1
            red = []
            idx = 0
            while idx < len(comb):
                if idx + 1 < len(comb) and comb[idx] == comb[idx + 1]:
                    idx += 2
                else:
                    red.append(comb[idx])
                    idx += 1
            S[i, j, b2i[tuple(red)]] = sign
    return S


# ------------------------------------------- affine box cover of the terms
def _box4_assign(tset):
    for split in combinations(range(4), 2):
        g1 = [tset[x] for x in split]
        g2 = [tset[x] for x in range(4) if x not in split]
        for p1 in permutations(g1):
            d1 = (p1[1][0] - p1[0][0], p1[1][1] - p1[0][1])
            for p2 in permutations(g2):
                d2 = (p2[1][0] - p2[0][0], p2[1][1] - p2[0][1])
                if d1 == d2:
                    return [p1[0], p1[1], p2[0], p2[1]]
    return None


def _cover_group(grp):
    best = None

    def rec(rem, acc):
        nonlocal best
        if len(rem) < 4:
            boxes = list(acc)
            r = list(rem)
            while len(r) >= 2:
                boxes.append([r[0], r[1]])
                r = r[2:]
            if r:
                boxes.append([r[0]])
            if best is None or len(boxes) < len(best):
                best = boxes
            return
        found4 = False
        for sub in combinations(range(len(rem)), 4):
            tset = [rem[x] for x in sub]
            a = _box4_assign(tset)
            if a:
                found4 = True
                rec([rem[x] for x in range(len(rem)) if x not in sub], acc + [a])
        if not found4:
            boxes = list(acc)
            r = list(rem)
            while len(r) >= 2:
                boxes.append([r[0], r[1]])
                r = r[2:]
            if r:
                boxes.append([r[0]])
            if best is None or len(boxes) < len(best):
                best = boxes

    rec(grp, [])
    return best


def _gen_ops(kd):
    """Product-op table. Each op: (sign, c1, c2, a_aff, b_aff, slot_aff, region)
    where *_aff = (offset, d1, d0) over a (c1 x c2) beta grid, slot indexes the
    region's product tile ([region-local k] * 8 + rank), region 0 = k<kd (DVE),
    region 1 = k>=kd (GPSIMD)."""
    S = _build_S()
    boxes = []
    for k in range(8):
        for sign in (1, -1):
            grp = [(i, j) for i in range(8) for j in range(8) if S[i, j, k] == sign]
            if not grp:
                continue
            for b in _cover_group(grp):
                boxes.append(dict(sign=sign, pairs=[(k, i, j) for (i, j) in b]))

    def region(k):
        return 0 if k < kd else 1

    # merge 2-boxes with equal (di, dj) deltas, same sign, same region
    twos = [b for b in boxes if len(b["pairs"]) == 2]
    others = [b for b in boxes if len(b["pairs"]) != 2]
    used = [False] * len(twos)
    merged = []
    for x in range(len(twos)):
        if used[x]:
            continue
        bx = twos[x]
        dx = tuple(np.subtract(bx["pairs"][1][1:], bx["pairs"][0][1:]))
        mx = None
        for y in range(x + 1, len(twos)):
            if used[y] or twos[y]["sign"] != bx["sign"]:
                continue
            if region(twos[y]["pairs"][0][0]) != region(bx["pairs"][0][0]):
                continue
            dy = tuple(np.subtract(twos[y]["pairs"][1][1:], twos[y]["pairs"][0][1:]))
            if dx == dy:
                mx = y
                break
        used[x] = True
        if mx is not None:
            used[mx] = True
            merged.append(dict(sign=bx["sign"], pairs=bx["pairs"] + twos[mx]["pairs"]))
        else:
            merged.append(bx)

    final = others + merged
    next_r = {k: 0 for k in range(8)}

    def slot(k, r):
        kk = k if k < kd else k - kd
        return kk * 8 + r

    ops = []
    for b in final:
        prs = b["pairs"]
        n = len(prs)
        if n == 4:
            k_a, k_b = prs[0][0], prs[2][0]
            ra = next_r[k_a]; next_r[k_a] += 2
            rb = next_r[k_b]; next_r[k_b] += 2
            slots = [slot(k_a, ra), slot(k_a, ra + 1), slot(k_b, rb), slot(k_b, rb + 1)]
            c1, c2 = 2, 2
        elif n == 2:
            k_a = prs[0][0]
            ra = next_r[k_a]; next_r[k_a] += 2
            slots = [slot(k_a, ra), slot(k_a, ra + 1)]
            c1, c2 = 1, 2
        else:
            k_a = prs[0][0]
            ra = next_r[k_a]; next_r[k_a] += 1
            slots = [slot(k_a, ra)]
            c1, c2 = 1, 1

        def aff(vals):
            if len(vals) == 1:
                return (vals[0], 0, 0)
            if len(vals) == 2:
                return (vals[0], 0, vals[1] - vals[0])
            o = vals[0]
            d0 = vals[1] - vals[0]
            d1 = vals[2] - vals[0]
            assert vals[3] == o + d0 + d1
            return (o, d1, d0)

        ops.append((
            b["sign"], c1, c2,
            aff([p[1] for p in prs]),
            aff([p[2] for p in prs]),
            aff(slots),
            region(prs[0][0]),
        ))
    assert all(v == 8 for v in next_r.values())
    # The NEFF verifier restricts ScalarTensorTensor (used for sign=-1) to
    # <=3D APs (partition + 2 free dims); split negative 4-boxes into 2-boxes.
    out_ops = []
    for (sign, c1, c2, a, b, s, reg) in ops:
        if sign == -1 and c1 == 2:
            for b1 in range(2):
                out_ops.append((
                    sign, 1, c2,
                    (a[0] + a[1] * b1, 0, a[2]),
                    (b[0] + b[1] * b1, 0, b[2]),
                    (s[0] + s[1] * b1, 0, s[2]),
                    reg,
                ))
        else:
            out_ops.append((sign, c1, c2, a, b, s, reg))
    return out_ops


# ------------------------------------------------------------ bass builder
def _mkap(base, dims, offset):
    """Custom free-dim AP over an SBUF tile AP: dims = [(stride, count), ...]."""
    ap = base.copy()
    part = list(base.ap[0])
    ap.ap = mybir.VecI64Pair([part] + [[d, c] for (d, c) in dims])
    ap.offset = base.offset + offset
    return ap


def build_nc(nc_mv=NC, e=E, kd=KD):
    n_tiles = nc_mv // (P * e)
    assert n_tiles * P * e == nc_mv
    ops = _gen_ops(kd)
    kg = 8 - kd                      # gpsimd component count
    w0, w1 = kd * 8, kg * 8          # product-tile slots per mv per region

    nc = bacc.Bacc("TRN2", target_bir_lowering=False, debug=False)
    ab_d = nc.dram_tensor("ab", [nc_mv, 16], F16, kind="ExternalInput")
    o_d = nc.dram_tensor("o", [nc_mv, 8], F16, kind="ExternalOutput")

    ab_v = ab_d.ap().rearrange("(t p e) c -> t p (e c)", t=n_tiles, p=P)
    o_v = o_d.ap().rearrange("(t p e) c -> t p (e c)", t=n_tiles, p=P)

    mult = mybir.AluOpType.mult
    add = mybir.AluOpType.add

    with TileContext(nc) as tc:
        with (
            tc.tile_pool(name="io", bufs=2) as io_pool,
            tc.tile_pool(name="prod", bufs=2) as prod_pool,
        ):
            for t in range(n_tiles):
                ab_t = io_pool.tile([P, 16 * e], F16, tag="ab")
                o_t = io_pool.tile([P, 8 * e], F16, tag="o")
                pd_t = prod_pool.tile([P, w0 * e], F16, tag="pd")
                if w1 > 0:
                    pg_t = prod_pool.tile([P, w1 * e], F16, tag="pg")
                else:
                    pg_t = pd_t

                # One dma_start for the packed tensor: a single InstDMACopy
                # is split across all 16 SDMA engines by the runtime.
                nc.sync.dma_start(out=ab_t[:, :], in_=ab_v[t])

                # ---- products ----
                for (sign, c1, c2, (ao, ad1, ad0), (bo, bd1, bd0),
                     (so, sd1, sd0), reg) in ops:
                    p_t, w = (pd_t, w0) if reg == 0 else (pg_t, w1)
                    dims_a = [(16, e), (ad1, c1), (ad0, c2)]
                    dims_b = [(16, e), (bd1, c1), (bd0, c2)]
                    dims_s = [(w, e), (sd1, c1), (sd0, c2)]
                    in0 = _mkap(ab_t, dims_a, ao)
                    in1 = _mkap(ab_t, dims_b, 8 + bo)
                    out = _mkap(p_t, dims_s, so)
                    if sign == 1:
                        nc.vector.tensor_tensor(out=out, in0=in0, in1=in1, op=mult)
                    else:
                        nc.vector.scalar_tensor_tensor(
                            out=out, in0=in0, scalar=-1.0, in1=in1,
                            op0=mult, op1=mult)

                # ---- reduction trees ----
                def tree(eng, p_t, w, nk, k0):
                    # L1: slots i<4 += i>=4 ; L2: i<2 += i in 2:4 ; L3 -> o_t
                    eng.tensor_tensor(
                        out=_mkap(p_t, [(w, e), (8, nk), (1, 4)], 0),
                        in0=_mkap(p_t, [(w, e), (8, nk), (1, 4)], 0),
                        in1=_mkap(p_t, [(w, e), (8, nk), (1, 4)], 4),
                        op=add)
                    eng.tensor_tensor(
                        out=_mkap(p_t, [(w, e), (8, nk), (1, 2)], 0),
                        in0=_mkap(p_t, [(w, e), (8, nk), (1, 2)], 0),
                        in1=_mkap(p_t, [(w, e), (8, nk), (1, 2)], 2),
                        op=add)
                    eng.tensor_tensor(
                        out=_mkap(o_t, [(8, e), (1, nk)], k0),
                        in0=_mkap(p_t, [(w, e), (8, nk)], 0),
                        in1=_mkap(p_t, [(w, e), (8, nk)], 1),
                        op=add)

                tree(nc.vector, pd_t, w0, kd, 0)
                if kg > 0:
                    tree(nc.gpsimd, pg_t, w1, kg, kd)

                nc.sync.dma_start(out=o_v[t], in_=o_t[:, :])
    nc.compile()
    return nc


_NC_CACHE = {}


def _get_nc(nc_mv, e, kd):
    key = (nc_mv, e, kd)
    if key not in _NC_CACHE:
        _NC_CACHE[key] = build_nc(nc_mv, e, kd)
    return _NC_CACHE[key]


def _pack_f16(a, b):
    """[n,8] f32 x2 -> [n,16] f16 packed, multithreaded cast."""
    n = a.shape[0]
    ab = np.empty((n, 16), np.float16)
    nch = 16
    step = (n + nch - 1) // nch

    def work(c):
        lo, hi = c * step, min((c + 1) * step, n)
        ab[lo:hi, :8] = a[lo:hi]
        ab[lo:hi, 8:] = b[lo:hi]

    list(_POOL.map(work, range(nch)))
    return ab


def _widen_f32(parts, nc_mv):
    """list of [nc_mv,8] f16 -> [n,8] f32, multithreaded cast."""
    out = np.empty((len(parts) * nc_mv, 8), np.float32)

    def work(c):
        out[c * nc_mv:(c + 1) * nc_mv] = parts[c]

    list(_POOL.map(work, range(len(parts))))
    return out


def kernel(a, b, M=None, **_):
    a = np.asarray(a)
    b = np.asarray(b)
    n = a.shape[0]
    assert n % N_CORES == 0
    nc_mv = n // N_CORES
    nc = _get_nc(nc_mv, E, KD)
    ab = _pack_f16(a, b)
    ab_sh = ab.reshape(N_CORES, nc_mv, 16)
    in_maps = [{"ab": ab_sh[c]} for c in range(N_CORES)]
    res = bass_utils.run_bass_kernel_spmd(nc, in_maps, core_ids=list(range(N_CORES)))
    return _widen_f32([r["o"].reshape(nc_mv, 8) for r in res.results], nc_mv)
